# revision 17
# baseline (speedup 1.0000x reference)
"""3-layer GCN (ColorGNN) on 8 Trainium2 NeuronCores.

Strategy (sharding_hint: shard nodes + incident edges, replicate weights):
  - Each core owns a contiguous slice of 1250 dst nodes. Edges (incl.
    self-loops) are bucketed by dst into per-core windows of 128 dst nodes,
    padded on host to a uniform chunk grid so all 8 cores run one SPMD
    instruction stream.
  - GCN normalization dinv[s]*dinv[d] is folded into host-built one-hot
    scatter matrices (edge->dst-slot), dinv^2 scales at the table
    evacuations, and rank-1 bias matmuls, so the per-layer pipeline is:
        gather rows (dma_gather from a replicated DRAM table)
        -> PE scatter-matmul into PSUM (feature-major, no transposes)
        -> relu / weight matmuls -> write next table slice
        -> AllGather table across cores.
  - Aggregation happens at the cheap end of each layer: layer 1 aggregates
    x at width 128 before W1; layers 2/3 aggregate after W2/W3 at widths
    256/64 (linearity of the GCN propagation).
"""

import sys

if "/opt/trn_rl_repo" not in sys.path:
    sys.path.insert(0, "/opt/trn_rl_repo")

import dataclasses

import numpy as np
import ml_dtypes

import concourse.bacc as bacc
import concourse.mybir as mybir
import concourse.tile as tile
from concourse.bass_utils import run_bass_kernel_spmd

# ---- problem constants (hardcoded per harness contract) ----
N = 10000
FEAT = 128
F1, F2, F3, FC = 512, 256, 64, 3
N_CORES = 8
SLICE = N // N_CORES          # 1250 dst nodes per core
W = 128                       # dst-window width (PSUM partition dim)
NW = (SLICE + W - 1) // W     # 10 windows; last is 98 wide
LAST_W = SLICE - (NW - 1) * W # 98

BF16 = mybir.dt.bfloat16
F32 = mybir.dt.float32
I16 = mybir.dt.int16

_cache = {}


# --------------------------------------------------------------------------
# host-side graph preprocessing (index/normalization structure only)
# --------------------------------------------------------------------------
def _preprocess(edge_index):
    src = np.asarray(edge_index[0], dtype=np.int64)
    dst = np.asarray(edge_index[1], dtype=np.int64)
    # self-loop edges are handled separately (contiguous local rows); drop
    # any explicit (i, i) duplicates from the edge list into the loop count.
    deg = np.bincount(dst, minlength=N).astype(np.float64) + 1.0
    dinv = 1.0 / np.sqrt(deg)

    keep = src != dst
    loop_extra = np.bincount(dst[~keep], minlength=N)  # explicit self-edges
    s, d = src[keep], dst[keep]

    core_of = d // SLICE
    win_of = (d % SLICE) // W
    order = np.lexsort((s, win_of, core_of))
    s, d = s[order], d[order]
    core_of, win_of = core_of[order], win_of[order]

    # per (core, window): dedup sources -> slots; S column = multi-hot counts
    run_key = (core_of * NW + win_of)
    run_starts = np.searchsorted(run_key, np.arange(N_CORES * NW))
    run_ends = np.append(run_starts[1:], len(s))

    # split each window's deduped sources into two groups by which half of
    # the split-layout tables (first 8*AGH rows vs rest) they live in, so
    # layer-2/3 gathers for group A can start after the first AllGather half.
    AGH = 640
    ACUT = N_CORES * AGH - 2   # last row is reserved for L3's 2-row reads

    def remap(g):
        gc, gi = g // SLICE, g % SLICE
        return np.where(gi < AGH, gc * AGH + gi,
                        N_CORES * AGH + gc * (SLICE - AGH) + (gi - AGH))

    slots_list = [[None] * NW for _ in range(N_CORES)]
    nslotA = np.zeros((N_CORES, NW), dtype=np.int64)
    nslotB = np.zeros((N_CORES, NW), dtype=np.int64)
    for c in range(N_CORES):
        for w_ in range(NW):
            k = c * NW + w_
            ss = s[run_starts[k]:run_ends[k]]
            dd = d[run_starts[k]:run_ends[k]]
            uniq, inv = np.unique(ss, return_inverse=True)
            is_a = remap(uniq) <= ACUT
            # stable reorder: A slots first, then B
            order_ = np.argsort(~is_a, kind="stable")
            rank = np.empty_like(order_)
            rank[order_] = np.arange(len(uniq))
            slots_list[c][w_] = (uniq[order_], rank[inv], dd, int(is_a.sum()))
            nslotA[c, w_] = is_a.sum()
            nslotB[c, w_] = len(uniq) - is_a.sum()
    CWA = int(np.max((nslotA + 127) // 128))
    CWB = int(np.max((nslotB + 127) // 128))
    CW = CWA + CWB
    EPW = CW * 128
    NCH = NW * CW
    EP = NCH * 128

    g_src = np.zeros((N_CORES, EP), dtype=np.int64)
    s01 = np.zeros((N_CORES, 128, NCH, 128), dtype=np.float32)
    dsrc = np.zeros((N_CORES, 128, NCH), dtype=np.float32)
    for c in range(N_CORES):
        for w_ in range(NW):
            uniq, inv, dd, na = slots_list[c][w_]
            base = w_ * EPW
            # slot position: A slots at [0, na), B slots at [CWA*128, ...)
            pos = np.arange(len(uniq))
            pos = np.where(pos < na, pos, CWA * 128 + (pos - na))
            np.put(g_src[c], base + pos, uniq)
            slot = base + pos[inv]
            part = slot % 128
            chunk = slot // 128
            dstl = (dd % SLICE) - w_ * W
            np.add.at(s01[c], (part, chunk, dstl), 1.0)
            sl = base + pos
            dsrc[c, sl % 128, sl // 128] = dinv[uniq]

    # gather idx layouts: wrapped-16 int16, one raw (L1/x) one remapped
    g2 = remap(g_src)
    gidx = np.zeros((N_CORES, 128, EP // 16), dtype=np.int16)
    gidx2 = np.zeros((N_CORES, 128, EP // 16), dtype=np.int16)
    for c in range(N_CORES):
        gidx[c] = np.tile(g_src[c].astype(np.int16).reshape(-1, 16).T, (8, 1))
        gidx2[c] = np.tile(g2[c].astype(np.int16).reshape(-1, 16).T, (8, 1))

    # per-window per-dst-node vectors
    dinv_pad = np.zeros((N_CORES, NW * W), dtype=np.float64)
    recip_pad = np.zeros((N_CORES, NW * W), dtype=np.float64)
    for c in range(N_CORES):
        sl = dinv[c * SLICE:(c + 1) * SLICE]
        dinv_pad[c, :SLICE] = sl
        recip_pad[c, :SLICE] = 1.0 / sl
    dinvT = np.ascontiguousarray(
        dinv_pad.reshape(N_CORES, NW, W).transpose(0, 2, 1)).astype(np.float32)
    dinv2T = np.ascontiguousarray(
        (dinv_pad ** 2).reshape(N_CORES, NW, W).transpose(0, 2, 1)).astype(np.float32)
    recip_row = recip_pad.astype(np.float32).reshape(N_CORES, 1, NW * W)

    # self-loop diagonal (value = dinv[n] * loop multiplicity incl implicit)
    diag = np.zeros((N_CORES, 128, NW * W), dtype=np.float32)
    loopv = dinv * (1.0 + loop_extra)
    for c in range(N_CORES):
        for w_ in range(NW):
            r = W if w_ < NW - 1 else LAST_W
            rows = np.arange(r)
            diag[c, rows, w_ * W + rows] = loopv[c * SLICE + w_ * W:
                                                 c * SLICE + w_ * W + r]
    # loop multiplicity for L2/L3 identity paths (value = multiplicity)
    diagc = np.zeros((N_CORES, 128, NW * W), dtype=np.float32)
    for c in range(N_CORES):
        for w_ in range(NW):
            r = W if w_ < NW - 1 else LAST_W
            rows = np.arange(r)
            diagc[c, rows, w_ * W + rows] = (
                1.0 + loop_extra[c * SLICE + w_ * W:c * SLICE + w_ * W + r])

    return dict(CW=CW, CWA=CWA, NCH=NCH, EP=EP, s01=s01, gidx=gidx,
                gidx2=gidx2, dsrc=dsrc, dinvT=dinvT, dinv2T=dinv2T,
                recip=recip_row, diag=diag, diagc=diagc)


# --------------------------------------------------------------------------
# device graph (one SPMD program for all 8 cores)
# --------------------------------------------------------------------------
def _build(CW, CWA, NCH, EP):
    # default 16KB SWDGE descriptor carveout -> 1024-descriptor ring per
    # queue; gather calls are split into <=GS-chunk pieces (GS*128
    # descriptors) and alternate between 2 queues so two stay in flight.
    nc = bacc.Bacc("TRN2", target_bir_lowering=False, debug=False,
                   num_swdge_queues=2)
    GS = 8
    AGH = 5 * W   # all-gather first-half rows (windows 0-4)

    xin = nc.dram_tensor("x", [N, FEAT], F32, kind="ExternalInput")
    w1d = nc.dram_tensor("W1", [FEAT, F1], F32, kind="ExternalInput")
    w2d = nc.dram_tensor("W2", [F1, F2], F32, kind="ExternalInput")
    w3d = nc.dram_tensor("W3", [F2, F3], F32, kind="ExternalInput")
    wcd = nc.dram_tensor("Wc", [F3, FC], F32, kind="ExternalInput")
    b1d = nc.dram_tensor("b1", [1, F1], F32, kind="ExternalInput")
    b2d = nc.dram_tensor("b2", [1, F2], F32, kind="ExternalInput")
    b3d = nc.dram_tensor("b3", [1, F3], F32, kind="ExternalInput")
    bcd = nc.dram_tensor("bc", [1, FC], F32, kind="ExternalInput")
    s01d = nc.dram_tensor("s01", [128, NCH * 128], BF16, kind="ExternalInput")
    gixd = nc.dram_tensor("gidx", [128, EP // 16], I16, kind="ExternalInput")
    gixd2 = nc.dram_tensor("gidx2", [128, EP // 16], I16, kind="ExternalInput")
    dsrcd = nc.dram_tensor("dsrc", [128, NCH], F32, kind="ExternalInput")
    dinvTd = nc.dram_tensor("dinvT", [128, NW], F32, kind="ExternalInput")
    dinv2Td = nc.dram_tensor("dinv2T", [128, NW], F32, kind="ExternalInput")
    recipd = nc.dram_tensor("recip", [1, NW * W], F32, kind="ExternalInput")
    diagd = nc.dram_tensor("diag", [128, NW * W], F32, kind="ExternalInput")
    diagcd = nc.dram_tensor("diagc", [128, NW * W], F32, kind="ExternalInput")
    diagcbd = nc.dram_tensor("diagcb", [128, NW * W], BF16, kind="ExternalInput")
    xselfd = nc.dram_tensor("xself", [NW * W, FEAT], F32, kind="ExternalInput")
    outd = nc.dram_tensor("out", [SLICE, FC], F32, kind="ExternalOutput")
    # +16 pad rows: L3 gathers read two consecutive 64-elem rows (512B
    # descriptors run ~2.7x faster than 256B), discarding the second row.
    t2_full = nc.dram_tensor("t2_full", [N, F2], BF16, kind="Internal",
                             addr_space="Shared")
    t3_full = nc.dram_tensor("t3_full", [N + 16, F3], F32, kind="Internal",
                             addr_space="Shared")

    RG = [list(range(N_CORES))]

    with tile.TileContext(nc) as tc:
        with (
            tc.tile_pool(name="res", bufs=1) as res,
            tc.tile_pool(name="msgs", bufs=6) as msgsp,
            tc.tile_pool(name="smat", bufs=2) as smatp,
            tc.tile_pool(name="ht", bufs=6) as htp,
            tc.tile_pool(name="evac", bufs=3) as evacp,
            tc.tile_pool(name="selfp", bufs=3) as selfp,
            tc.tile_pool(name="pz", bufs=2, space="PSUM") as pzp,
            tc.tile_pool(name="ph", bufs=3, space="PSUM") as php,
            tc.tile_pool(name="py", bufs=2, space="PSUM") as pyp,
            tc.tile_pool(name="dram", bufs=1, space="DRAM") as dram,
        ):
            # ---- PE warm-up: dense dummy matmuls so HAM unthrottles before
            # layer 1 (overlaps the input loads / entry barrier) ----
            wu = res.tile([128, 512], F32)
            nc.gpsimd.memset(wu[:], 1.0)
            wups = pyp.tile([128, 512], F32, tag="py")
            for i in range(14):
                nc.tensor.matmul(wups[:], wu[:, 0:128], wu[:],
                                 start=(i == 0), stop=(i == 13))
            wuev = res.tile([128, 4], F32)
            nc.vector.tensor_copy(wuev[:], wups[:, 0:4])
            nc.sync.dma_start(outd[0:128, 0:3], wuev[:, 0:3])

            # ---- resident loads ----
            s01 = res.tile([128, NCH, 128], BF16)
            nc.sync.dma_start(s01[:], s01d[:].rearrange("p (c j) -> p c j", j=128))
            gix = res.tile([128, EP // 16], I16)
            nc.sync.dma_start(gix[:], gixd[:])
            gix2 = res.tile([128, EP // 16], I16)
            nc.sync.dma_start(gix2[:], gixd2[:])
            dsrc = res.tile([128, NCH], F32)
            nc.sync.dma_start(dsrc[:], dsrcd[:])
            dinvT = res.tile([128, NW], F32)
            nc.sync.dma_start(dinvT[:], dinvTd[:])
            dinv2T = res.tile([128, NW], F32)
            nc.sync.dma_start(dinv2T[:], dinv2Td[:])
            recip = res.tile([1, NW * W], F32)
            nc.sync.dma_start(recip[:], recipd[:])
            diag = res.tile([128, NW * W], F32)
            nc.sync.dma_start(diag[:], diagd[:])
            diagc = res.tile([128, NW * W], F32)
            nc.sync.dma_start(diagc[:], diagcd[:])
            diagcb = res.tile([128, NW * W], BF16)
            nc.sync.dma_start(diagcb[:], diagcbd[:])
            w1 = res.tile([128, F1], F32)
            nc.sync.dma_start(w1[:], w1d[:])
            w2 = res.tile([128, 4, F2], F32)
            nc.sync.dma_start(w2[:], w2d[:].rearrange("(c p) f -> p c f", p=128))
            w3 = res.tile([128, 2, F3], F32)
            nc.sync.dma_start(w3[:], w3d[:].rearrange("(c p) f -> p c f", p=128))
            wc = res.tile([F3, FC], F32)
            nc.sync.dma_start(wc[:], wcd[:])
            b1 = res.tile([1, F1], F32)
            nc.sync.dma_start(b1[:], b1d[:])
            b2 = res.tile([1, F2], F32)
            nc.sync.dma_start(b2[:], b2d[:])
            b3 = res.tile([1, F3], F32)
            nc.sync.dma_start(b3[:], b3d[:])
            bc = res.tile([1, FC], F32)
            nc.sync.dma_start(bc[:], bcd[:])

            # ---- internal DRAM tables ----
            t2_in = dram.tile([NW * W, F2], BF16)
            t3_in = dram.tile([NW * W, F3], F32)

            qctr = [0]

            def gather(dst_tile, table_ap, idx_tile, w_, elem, c0=0, c1=None,
                       elem_step=None):
                if c1 is None:
                    c1 = CW
                for a in range(c0, c1, GS):
                    b = min(a + GS, c1)
                    n_ = (b - a) * 128
                    nc.gpsimd.dma_gather(
                        dst_tile[:, a:b, :], table_ap,
                        idx_tile[:, (w_ * CW + a) * 8:(w_ * CW + b) * 8],
                        n_, n_, elem, elem_step=elem_step,
                        queue_num=qctr[0] % 2)
                    qctr[0] += 1

            def rows_of(w_):
                return W if w_ < NW - 1 else LAST_W

            def win(t, w_):  # [1, W] slice of a [1, NW*W] row vector
                return t[:, w_ * W:(w_ + 1) * W]

            # ================= layer 1 (aggregate x @ width 128) ==========
            for w_ in range(NW):
                r = rows_of(w_)
                msgs = msgsp.tile([128, CW, FEAT], F32, tag="msgs")
                gather(msgs, xin[:], gix, w_, FEAT)
                s1 = smatp.tile([128, CW, 128], F32, tag="smat")
                nc.vector.tensor_tensor(
                    s1[:], s01[:, w_ * CW:(w_ + 1) * CW, :],
                    dsrc[:, w_ * CW:(w_ + 1) * CW].to_broadcast((128, CW, 128)),
                    mybir.AluOpType.mult)
                xself = selfp.tile([128, FEAT], F32, tag="xself")
                nc.sync.dma_start(
                    xself[:r, :], xselfd[w_ * W:w_ * W + r, :])
                pz = pzp.tile([128, W], F32, tag="pz")
                for k in range(CW):
                    nc.tensor.matmul(pz[:], msgs[:, k, :], s1[:, k, :],
                                     start=(k == 0), stop=False)
                nc.tensor.matmul(pz[:], xself[:r, :],
                                 diag[0:r, w_ * W:(w_ + 1) * W],
                                 start=False, stop=True)
                z1 = evacp.tile([128, W], F32, tag="z1")
                nc.vector.tensor_copy(z1[:], pz[:])

                hts = []
                for c4 in range(4):
                    ph = php.tile([128, W], F32, tag="ph")
                    nc.tensor.matmul(ph[:], w1[:, c4 * 128:(c4 + 1) * 128], z1[:],
                                     start=True, stop=False)
                    nc.tensor.matmul(ph[:], b1[:, c4 * 128:(c4 + 1) * 128],
                                     win(recip, w_), start=False, stop=True)
                    ht = htp.tile([128, W], F32, tag="ht")
                    nc.scalar.activation(ht[:], ph[:],
                                         mybir.ActivationFunctionType.Relu)
                    hts.append(ht)
                py = pyp.tile([128, F2], F32, tag="py")
                for c4 in range(4):
                    nc.tensor.matmul(py[:], hts[c4][:], w2[:, c4, :],
                                     start=(c4 == 0), stop=(c4 == 3))
                y2 = evacp.tile([128, F2], BF16, tag="y2")
                nc.scalar.activation(y2[:], py[:],
                                     mybir.ActivationFunctionType.Copy,
                                     scale=dinv2T[:, w_:w_ + 1])
                nc.sync.dma_start(t2_in[w_ * W:w_ * W + r, :], y2[:r, :])
                if w_ == 7:
                    nc.gpsimd.collective_compute(
                        "AllGather", mybir.AluOpType.bypass,
                        ins=[t2_in[0:AGH, :]],
                        outs=[t2_full[0:N_CORES * AGH, :]], replica_groups=RG)
            nc.gpsimd.collective_compute(
                "AllGather", mybir.AluOpType.bypass,
                ins=[t2_in[AGH:SLICE, :]], outs=[t2_full[N_CORES * AGH:N, :]],
                replica_groups=RG)

            # ================= layer 2 (aggregate y2 @ width 256) =========
            # group-A gathers (sources in the first AG half) run LOOK windows
            # ahead so generation starts as soon as the first half lands.
            LOOK = 3
            t2_a = t2_full[0:N_CORES * AGH, :]
            l2_msgs = {}

            def l2_head(w_):
                msgs = msgsp.tile([128, CW, F2], BF16, tag="msgs")
                l2_msgs[w_] = msgs
                gather(msgs, t2_a, gix2, w_, F2, 0, CWA)

            def l2_body(w_):
                r = rows_of(w_)
                msgs = l2_msgs.pop(w_)
                gather(msgs, t2_full[:], gix2, w_, F2, CWA, CW)
                y2self = selfp.tile([128, F2], BF16, tag="y2self")
                nc.sync.dma_start(y2self[:r, :],
                                  t2_in[w_ * W:w_ * W + r, :])
                hts = []
                for m in range(2):
                    pz = pzp.tile([128, W], F32, tag="pz")
                    for k in range(CW):
                        nc.tensor.matmul(pz[:], msgs[:, k, m * 128:(m + 1) * 128],
                                         s01[:, w_ * CW + k, :],
                                         start=(k == 0), stop=False)
                    nc.tensor.matmul(pz[:], y2self[:r, m * 128:(m + 1) * 128],
                                     diagcb[0:r, w_ * W:(w_ + 1) * W],
                                     start=False, stop=False)
                    nc.tensor.matmul(pz[:], b2[:, m * 128:(m + 1) * 128],
                                     win(recip, w_), start=False, stop=True)
                    ht = htp.tile([128, W], F32, tag="ht")
                    nc.scalar.activation(ht[:], pz[:],
                                         mybir.ActivationFunctionType.Relu)
                    hts.append(ht)
                py = pyp.tile([128, F3], F32, tag="py")
                for m in range(2):
                    nc.tensor.matmul(py[:], hts[m][:], w3[:, m, :],
                                     start=(m == 0), stop=(m == 1))
                y3 = evacp.tile([128, F3], F32, tag="y3")
                nc.scalar.activation(y3[:], py[:],
                                     mybir.ActivationFunctionType.Copy,
                                     scale=dinv2T[:, w_:w_ + 1])
                nc.sync.dma_start(t3_in[w_ * W:w_ * W + r, :], y3[:r, :])
                if w_ == 4:
                    nc.gpsimd.collective_compute(
                        "AllGather", mybir.AluOpType.bypass,
                        ins=[t3_in[0:AGH, :]],
                        outs=[t3_full[0:N_CORES * AGH, :]], replica_groups=RG)

            for w_ in range(NW):
                l2_head(w_)
                if w_ >= LOOK:
                    l2_body(w_ - LOOK)
            for w_ in range(NW - LOOK, NW):
                l2_body(w_)
            nc.gpsimd.collective_compute(
                "AllGather", mybir.AluOpType.bypass,
                ins=[t3_in[AGH:SLICE, :]], outs=[t3_full[N_CORES * AGH:N, :]],
                replica_groups=RG)

            # ================= layer 3 (aggregate y3 @ width 64) ==========
            t3ov_a = dataclasses.replace(
                t3_full[:], ap=[[F3, N_CORES * AGH - 1], [1, 2 * F3]])
            t3ov = dataclasses.replace(t3_full[:], ap=[[F3, N], [1, 2 * F3]])
            l3_msgs = {}

            def l3_head(w_):
                msgs = msgsp.tile([128, CW, 2 * F3], F32, tag="msgs")
                l3_msgs[w_] = msgs
                gather(msgs, t3ov_a, gix2, w_, 2 * F3, 0, CWA, elem_step=F3)

            def l3_body(w_):
                r = rows_of(w_)
                msgs = l3_msgs.pop(w_)
                gather(msgs, t3ov, gix2, w_, 2 * F3, CWA, CW, elem_step=F3)
                s3 = smatp.tile([128, CW, 128], F32, tag="smat")
                nc.vector.tensor_copy(s3[:], s01[:, w_ * CW:(w_ + 1) * CW, :])
                y3self = selfp.tile([128, F3], F32, tag="y3self")
                nc.sync.dma_start(y3self[:r, :],
                                  t3_in[w_ * W:w_ * W + r, :])
                pz = pzp.tile([F3, W], F32, tag="pz")
                for k in range(CW):
                    nc.tensor.matmul(pz[:], msgs[:, k, 0:F3], s3[:, k, :],
                                     start=(k == 0), stop=False)
                nc.tensor.matmul(pz[:], y3self[:r, :],
                                 diagc[0:r, w_ * W:(w_ + 1) * W],
                                 start=False, stop=False)
                nc.tensor.matmul(pz[:], b3[:], win(recip, w_),
                                 start=False, stop=True)
                ht = htp.tile([F3, W], F32, tag="ht3")
                nc.scalar.activation(ht[:], pz[:],
                                     mybir.ActivationFunctionType.Relu)
                po = php.tile([128, FC], F32, tag="ph")
                nc.tensor.matmul(po[:], ht[:], wc[:], start=True, stop=False)
                nc.tensor.matmul(po[:], win(recip, w_), bc[:],
                                 start=False, stop=True)
                os_ = evacp.tile([128, FC], F32, tag="os")
                nc.scalar.activation(os_[:], po[:],
                                     mybir.ActivationFunctionType.Copy,
                                     scale=dinvT[:, w_:w_ + 1])
                nc.sync.dma_start(outd[w_ * W:w_ * W + r, :], os_[:r, :])

            for w_ in range(NW):
                l3_head(w_)
                if w_ >= LOOK:
                    l3_body(w_ - LOOK)
            for w_ in range(NW - LOOK, NW):
                l3_body(w_)

    nc.compile()
    return nc


# --------------------------------------------------------------------------
def kernel(x, W1, b1, W2, b2, W3, b3, Wc, bc, edge_index, _run_kwargs=None):
    x = np.asarray(x, dtype=np.float32)
    pre = _preprocess(np.asarray(edge_index))
    CW, NCH, EP = pre["CW"], pre["NCH"], pre["EP"]

    key = (CW, pre["CWA"])
    if key not in _cache:
        _cache[key] = _build(CW, pre["CWA"], NCH, EP)
    nc = _cache[key]

    common = {
        "x": x,
        "W1": np.asarray(W1, np.float32), "W2": np.asarray(W2, np.float32),
        "W3": np.asarray(W3, np.float32), "Wc": np.asarray(Wc, np.float32),
        "b1": np.asarray(b1, np.float32).reshape(1, F1),
        "b2": np.asarray(b2, np.float32).reshape(1, F2),
        "b3": np.asarray(b3, np.float32).reshape(1, F3),
        "bc": np.asarray(bc, np.float32).reshape(1, FC),
    }
    in_maps = []
    for c in range(N_CORES):
        m = dict(common)
        m["s01"] = pre["s01"][c].reshape(128, NCH * 128).astype(ml_dtypes.bfloat16)
        m["gidx"] = pre["gidx"][c]
        m["gidx2"] = pre["gidx2"][c]
        m["dsrc"] = pre["dsrc"][c]
        m["dinvT"] = pre["dinvT"][c]
        m["diag"] = pre["diag"][c]
        m["diagc"] = pre["diagc"][c]
        m["diagcb"] = pre["diagc"][c].astype(ml_dtypes.bfloat16)
        xs = np.zeros((NW * W, FEAT), np.float32)
        xs[:SLICE] = x[c * SLICE:(c + 1) * SLICE]
        m["xself"] = xs
        m["dinv2T"] = pre["dinv2T"][c]
        m["recip"] = pre["recip"][c]
        in_maps.append(m)

    kw = dict(_run_kwargs or {})
    res = run_bass_kernel_spmd(nc, in_maps, core_ids=list(range(N_CORES)), **kw)
    out = np.concatenate([res.results[c]["out"] for c in range(N_CORES)], axis=0)
    kernel._last_result = res
    return out


# revision 20
# speedup vs baseline: 1.0531x; 1.0531x over previous
"""3-layer GCN (ColorGNN) on 8 Trainium2 NeuronCores.

Strategy (sharding_hint: shard nodes + incident edges, replicate weights):
  - Each core owns a contiguous slice of 1250 dst nodes. Edges (incl.
    self-loops) are bucketed by dst into per-core windows of 128 dst nodes,
    padded on host to a uniform chunk grid so all 8 cores run one SPMD
    instruction stream.
  - GCN normalization dinv[s]*dinv[d] is folded into host-built one-hot
    scatter matrices (edge->dst-slot), dinv^2 scales at the table
    evacuations, and rank-1 bias matmuls, so the per-layer pipeline is:
        gather rows (dma_gather from a replicated DRAM table)
        -> PE scatter-matmul into PSUM (feature-major, no transposes)
        -> relu / weight matmuls -> write next table slice
        -> AllGather table across cores.
  - Aggregation happens at the cheap end of each layer: layer 1 aggregates
    x at width 128 before W1; layers 2/3 aggregate after W2/W3 at widths
    256/64 (linearity of the GCN propagation).
"""

import sys

if "/opt/trn_rl_repo" not in sys.path:
    sys.path.insert(0, "/opt/trn_rl_repo")

import dataclasses

import numpy as np
import ml_dtypes

import concourse.bacc as bacc
import concourse.mybir as mybir
import concourse.tile as tile
import concourse.tile_rust as tile_rust
from concourse.bass_utils import run_bass_kernel_spmd

# ---- problem constants (hardcoded per harness contract) ----
N = 10000
FEAT = 128
F1, F2, F3, FC = 512, 256, 64, 3
N_CORES = 8
SLICE = N // N_CORES          # 1250 dst nodes per core
W = 128                       # dst-window width (PSUM partition dim)
NW = (SLICE + W - 1) // W     # 10 windows; last is 98 wide
LAST_W = SLICE - (NW - 1) * W # 98

BF16 = mybir.dt.bfloat16
F32 = mybir.dt.float32
I16 = mybir.dt.int16

_cache = {}


# --------------------------------------------------------------------------
# host-side graph preprocessing (index/normalization structure only)
# --------------------------------------------------------------------------
def _preprocess(edge_index):
    src = np.asarray(edge_index[0], dtype=np.int64)
    dst = np.asarray(edge_index[1], dtype=np.int64)
    # self-loop edges are handled separately (contiguous local rows); drop
    # any explicit (i, i) duplicates from the edge list into the loop count.
    deg = np.bincount(dst, minlength=N).astype(np.float64) + 1.0
    dinv = 1.0 / np.sqrt(deg)

    keep = src != dst
    loop_extra = np.bincount(dst[~keep], minlength=N)  # explicit self-edges
    s, d = src[keep], dst[keep]

    core_of = d // SLICE
    win_of = (d % SLICE) // W
    order = np.lexsort((s, win_of, core_of))
    s, d = s[order], d[order]
    core_of, win_of = core_of[order], win_of[order]

    # per (core, window): dedup sources -> slots; S column = multi-hot counts
    run_key = (core_of * NW + win_of)
    run_starts = np.searchsorted(run_key, np.arange(N_CORES * NW))
    run_ends = np.append(run_starts[1:], len(s))

    # split each window's deduped sources into two groups by which half of
    # the split-layout tables (first 8*AGH rows vs rest) they live in, so
    # layer-2/3 gathers for group A can start after the first AllGather half.
    AGH = 640
    ACUT = N_CORES * AGH - 2   # last row is reserved for L3's 2-row reads

    def remap(g):
        gc, gi = g // SLICE, g % SLICE
        return np.where(gi < AGH, gc * AGH + gi,
                        N_CORES * AGH + gc * (SLICE - AGH) + (gi - AGH))

    slots_list = [[None] * NW for _ in range(N_CORES)]
    nslotA = np.zeros((N_CORES, NW), dtype=np.int64)
    nslotB = np.zeros((N_CORES, NW), dtype=np.int64)
    for c in range(N_CORES):
        for w_ in range(NW):
            k = c * NW + w_
            ss = s[run_starts[k]:run_ends[k]]
            dd = d[run_starts[k]:run_ends[k]]
            uniq, inv = np.unique(ss, return_inverse=True)
            is_a = remap(uniq) <= ACUT
            # stable reorder: A slots first, then B
            order_ = np.argsort(~is_a, kind="stable")
            rank = np.empty_like(order_)
            rank[order_] = np.arange(len(uniq))
            slots_list[c][w_] = (uniq[order_], rank[inv], dd, int(is_a.sum()))
            nslotA[c, w_] = is_a.sum()
            nslotB[c, w_] = len(uniq) - is_a.sum()
    CWA = int(np.max((nslotA + 127) // 128))
    CWB = int(np.max((nslotB + 127) // 128))
    CW = CWA + CWB
    EPW = CW * 128
    NCH = NW * CW
    EP = NCH * 128

    g_src = np.zeros((N_CORES, EP), dtype=np.int64)
    s01 = np.zeros((N_CORES, 128, NCH, 128), dtype=np.float32)
    dsrc = np.zeros((N_CORES, 128, NCH), dtype=np.float32)
    for c in range(N_CORES):
        for w_ in range(NW):
            uniq, inv, dd, na = slots_list[c][w_]
            base = w_ * EPW
            # slot position: A slots at [0, na), B slots at [CWA*128, ...)
            pos = np.arange(len(uniq))
            pos = np.where(pos < na, pos, CWA * 128 + (pos - na))
            np.put(g_src[c], base + pos, uniq)
            slot = base + pos[inv]
            part = slot % 128
            chunk = slot // 128
            dstl = (dd % SLICE) - w_ * W
            np.add.at(s01[c], (part, chunk, dstl), 1.0)
            sl = base + pos
            dsrc[c, sl % 128, sl // 128] = dinv[uniq]

    # gather idx layouts: wrapped-16 int16, one raw (L1/x) one remapped
    g2 = remap(g_src)
    gidx = np.zeros((N_CORES, 128, EP // 16), dtype=np.int16)
    gidx2 = np.zeros((N_CORES, 128, EP // 16), dtype=np.int16)
    for c in range(N_CORES):
        gidx[c] = np.tile(g_src[c].astype(np.int16).reshape(-1, 16).T, (8, 1))
        gidx2[c] = np.tile(g2[c].astype(np.int16).reshape(-1, 16).T, (8, 1))

    # per-window per-dst-node vectors
    dinv_pad = np.zeros((N_CORES, NW * W), dtype=np.float64)
    recip_pad = np.zeros((N_CORES, NW * W), dtype=np.float64)
    for c in range(N_CORES):
        sl = dinv[c * SLICE:(c + 1) * SLICE]
        dinv_pad[c, :SLICE] = sl
        recip_pad[c, :SLICE] = 1.0 / sl
    dinvT = np.ascontiguousarray(
        dinv_pad.reshape(N_CORES, NW, W).transpose(0, 2, 1)).astype(np.float32)
    dinv2T = np.ascontiguousarray(
        (dinv_pad ** 2).reshape(N_CORES, NW, W).transpose(0, 2, 1)).astype(np.float32)
    recip_row = recip_pad.astype(np.float32).reshape(N_CORES, 1, NW * W)

    # self-loop diagonal (value = dinv[n] * loop multiplicity incl implicit)
    diag = np.zeros((N_CORES, 128, NW * W), dtype=np.float32)
    loopv = dinv * (1.0 + loop_extra)
    for c in range(N_CORES):
        for w_ in range(NW):
            r = W if w_ < NW - 1 else LAST_W
            rows = np.arange(r)
            diag[c, rows, w_ * W + rows] = loopv[c * SLICE + w_ * W:
                                                 c * SLICE + w_ * W + r]
    # loop multiplicity for L2/L3 identity paths (value = multiplicity)
    diagc = np.zeros((N_CORES, 128, NW * W), dtype=np.float32)
    for c in range(N_CORES):
        for w_ in range(NW):
            r = W if w_ < NW - 1 else LAST_W
            rows = np.arange(r)
            diagc[c, rows, w_ * W + rows] = (
                1.0 + loop_extra[c * SLICE + w_ * W:c * SLICE + w_ * W + r])

    return dict(CW=CW, CWA=CWA, NCH=NCH, EP=EP, s01=s01, gidx=gidx,
                gidx2=gidx2, dsrc=dsrc, dinvT=dinvT, dinv2T=dinv2T,
                recip=recip_row, diag=diag, diagc=diagc)


# --------------------------------------------------------------------------
# device graph (one SPMD program for all 8 cores)
# --------------------------------------------------------------------------
def _build(CW, CWA, NCH, EP):
    # default 16KB SWDGE descriptor carveout -> 1024-descriptor ring per
    # queue; gather calls are split into <=GS-chunk pieces (GS*128
    # descriptors) and alternate between 2 queues so two stay in flight.
    nc = bacc.Bacc("TRN2", target_bir_lowering=False, debug=False,
                   num_swdge_queues=2)
    GS = 8
    AGH = 5 * W   # all-gather first-half rows (windows 0-4)

    xin = nc.dram_tensor("x", [N, FEAT], F32, kind="ExternalInput")
    w1d = nc.dram_tensor("W1", [FEAT, F1], F32, kind="ExternalInput")
    w2d = nc.dram_tensor("W2", [F1, F2], F32, kind="ExternalInput")
    w3d = nc.dram_tensor("W3", [F2, F3], F32, kind="ExternalInput")
    wcd = nc.dram_tensor("Wc", [F3, FC], F32, kind="ExternalInput")
    b1d = nc.dram_tensor("b1", [1, F1], F32, kind="ExternalInput")
    b2d = nc.dram_tensor("b2", [1, F2], F32, kind="ExternalInput")
    b3d = nc.dram_tensor("b3", [1, F3], F32, kind="ExternalInput")
    bcd = nc.dram_tensor("bc", [1, FC], F32, kind="ExternalInput")
    s01d = nc.dram_tensor("s01", [128, NCH * 128], BF16, kind="ExternalInput")
    gixd = nc.dram_tensor("gidx", [128, EP // 16], I16, kind="ExternalInput")
    gixd2 = nc.dram_tensor("gidx2", [128, EP // 16], I16, kind="ExternalInput")
    dsrcd = nc.dram_tensor("dsrc", [128, NCH], F32, kind="ExternalInput")
    dinvTd = nc.dram_tensor("dinvT", [128, NW], F32, kind="ExternalInput")
    dinv2Td = nc.dram_tensor("dinv2T", [128, NW], F32, kind="ExternalInput")
    recipd = nc.dram_tensor("recip", [1, NW * W], F32, kind="ExternalInput")
    diagd = nc.dram_tensor("diag", [128, NW * W], F32, kind="ExternalInput")
    diagcd = nc.dram_tensor("diagc", [128, NW * W], F32, kind="ExternalInput")
    diagcbd = nc.dram_tensor("diagcb", [128, NW * W], BF16, kind="ExternalInput")
    xselfd = nc.dram_tensor("xself", [NW * W, FEAT], F32, kind="ExternalInput")
    outd = nc.dram_tensor("out", [SLICE, FC], F32, kind="ExternalOutput")
    # +16 pad rows: L3 gathers read two consecutive 128-elem bf16 rows
    # (512B descriptors run ~2.7x faster than 256B), discarding the pad
    # halves; bf16 rows let the scatter use s01 directly (no DVE cast,
    # which locks GpSimd out of the SWDGE descriptor rings).
    t2_full = nc.dram_tensor("t2_full", [N, F2], BF16, kind="Internal",
                             addr_space="Shared")
    t3_full = nc.dram_tensor("t3_full", [N + 16, 2 * F3], BF16,
                             kind="Internal", addr_space="Shared")

    RG = [list(range(N_CORES))]

    with tile.TileContext(nc) as tc:
        with (
            tc.tile_pool(name="res", bufs=1) as res,
            tc.tile_pool(name="msgs", bufs=6) as msgsp,
            tc.tile_pool(name="smat", bufs=2) as smatp,
            tc.tile_pool(name="ht", bufs=6) as htp,
            tc.tile_pool(name="evac", bufs=3) as evacp,
            tc.tile_pool(name="selfp", bufs=3) as selfp,
            tc.tile_pool(name="pz", bufs=2, space="PSUM") as pzp,
            tc.tile_pool(name="ph", bufs=3, space="PSUM") as php,
            tc.tile_pool(name="py", bufs=2, space="PSUM") as pyp,
            tc.tile_pool(name="dram", bufs=1, space="DRAM") as dram,
        ):
            # ---- PE warm-up: dense dummy matmuls so HAM unthrottles before
            # layer 1 (overlaps the input loads / entry barrier) ----
            wu = res.tile([128, 512], F32)
            nc.gpsimd.memset(wu[:], 1.0)
            wups = pyp.tile([128, 512], F32, tag="py")
            for i in range(14):
                nc.tensor.matmul(wups[:], wu[:, 0:128], wu[:],
                                 start=(i == 0), stop=(i == 13))
            wuev = res.tile([128, 4], F32)
            nc.vector.tensor_copy(wuev[:], wups[:, 0:4])
            nc.sync.dma_start(outd[0:128, 0:3], wuev[:, 0:3])

            # ---- resident loads ----
            s01 = res.tile([128, NCH, 128], BF16)
            nc.sync.dma_start(s01[:], s01d[:].rearrange("p (c j) -> p c j", j=128))
            gix = res.tile([128, EP // 16], I16)
            nc.sync.dma_start(gix[:], gixd[:])
            gix2 = res.tile([128, EP // 16], I16)
            nc.sync.dma_start(gix2[:], gixd2[:])
            dsrc = res.tile([128, NCH], F32)
            nc.sync.dma_start(dsrc[:], dsrcd[:])
            dinvT = res.tile([128, NW], F32)
            nc.sync.dma_start(dinvT[:], dinvTd[:])
            dinv2T = res.tile([128, NW], F32)
            nc.sync.dma_start(dinv2T[:], dinv2Td[:])
            recip = res.tile([1, NW * W], F32)
            nc.sync.dma_start(recip[:], recipd[:])
            diag = res.tile([128, NW * W], F32)
            nc.sync.dma_start(diag[:], diagd[:])
            diagc = res.tile([128, NW * W], F32)
            nc.sync.dma_start(diagc[:], diagcd[:])
            diagcb = res.tile([128, NW * W], BF16)
            nc.sync.dma_start(diagcb[:], diagcbd[:])
            w1 = res.tile([128, F1], F32)
            nc.sync.dma_start(w1[:], w1d[:])
            w2 = res.tile([128, 4, F2], F32)
            nc.sync.dma_start(w2[:], w2d[:].rearrange("(c p) f -> p c f", p=128))
            w3 = res.tile([128, 2, F3], F32)
            nc.sync.dma_start(w3[:], w3d[:].rearrange("(c p) f -> p c f", p=128))
            wc = res.tile([F3, FC], F32)
            nc.sync.dma_start(wc[:], wcd[:])
            b1 = res.tile([1, F1], F32)
            nc.sync.dma_start(b1[:], b1d[:])
            b2 = res.tile([1, F2], F32)
            nc.sync.dma_start(b2[:], b2d[:])
            b3 = res.tile([1, F3], F32)
            nc.sync.dma_start(b3[:], b3d[:])
            bc = res.tile([1, FC], F32)
            nc.sync.dma_start(bc[:], bcd[:])

            # ---- internal DRAM tables ----
            t2_in = dram.tile([NW * W, F2], BF16)
            t3_in = dram.tile([NW * W, 2 * F3], BF16)

            qctr = [0]

            def gather(dst_tile, table_ap, idx_tile, w_, elem, c0=0, c1=None,
                       elem_step=None):
                insts = []
                if c1 is None:
                    c1 = CW
                for a in range(c0, c1, GS):
                    b = min(a + GS, c1)
                    n_ = (b - a) * 128
                    insts.append(nc.gpsimd.dma_gather(
                        dst_tile[:, a:b, :], table_ap,
                        idx_tile[:, (w_ * CW + a) * 8:(w_ * CW + b) * 8],
                        n_, n_, elem, elem_step=elem_step,
                        queue_num=qctr[0] % 2))
                    qctr[0] += 1
                return insts

            def rows_of(w_):
                return W if w_ < NW - 1 else LAST_W

            def win(t, w_):  # [1, W] slice of a [1, NW*W] row vector
                return t[:, w_ * W:(w_ + 1) * W]

            # ================= layer 1 (aggregate x @ width 128) ==========
            for w_ in range(NW):
                r = rows_of(w_)
                msgs = msgsp.tile([128, CW, FEAT], F32, tag="msgs")
                gather(msgs, xin[:], gix, w_, FEAT)
                s1 = smatp.tile([128, CW, 128], F32, tag="smat")
                nc.vector.tensor_tensor(
                    s1[:], s01[:, w_ * CW:(w_ + 1) * CW, :],
                    dsrc[:, w_ * CW:(w_ + 1) * CW].to_broadcast((128, CW, 128)),
                    mybir.AluOpType.mult)
                xself = selfp.tile([128, FEAT], F32, tag="xself")
                nc.sync.dma_start(
                    xself[:r, :], xselfd[w_ * W:w_ * W + r, :])
                pz = pzp.tile([128, W], F32, tag="pz")
                for k in range(CW):
                    nc.tensor.matmul(pz[:], msgs[:, k, :], s1[:, k, :],
                                     start=(k == 0), stop=False)
                nc.tensor.matmul(pz[:], xself[:r, :],
                                 diag[0:r, w_ * W:(w_ + 1) * W],
                                 start=False, stop=True)
                z1 = evacp.tile([128, W], F32, tag="z1")
                nc.vector.tensor_copy(z1[:], pz[:])

                hts = []
                for c4 in range(4):
                    ph = php.tile([128, W], F32, tag="ph")
                    nc.tensor.matmul(ph[:], w1[:, c4 * 128:(c4 + 1) * 128], z1[:],
                                     start=True, stop=False)
                    nc.tensor.matmul(ph[:], b1[:, c4 * 128:(c4 + 1) * 128],
                                     win(recip, w_), start=False, stop=True)
                    ht = htp.tile([128, W], F32, tag="ht")
                    nc.scalar.activation(ht[:], ph[:],
                                         mybir.ActivationFunctionType.Relu)
                    hts.append(ht)
                py = pyp.tile([128, F2], F32, tag="py")
                for c4 in range(4):
                    nc.tensor.matmul(py[:], hts[c4][:], w2[:, c4, :],
                                     start=(c4 == 0), stop=(c4 == 3))
                y2 = evacp.tile([128, F2], BF16, tag="y2")
                nc.scalar.activation(y2[:], py[:],
                                     mybir.ActivationFunctionType.Copy,
                                     scale=dinv2T[:, w_:w_ + 1])
                nc.sync.dma_start(t2_in[w_ * W:w_ * W + r, :], y2[:r, :])
                if w_ == 7:
                    nc.gpsimd.collective_compute(
                        "AllGather", mybir.AluOpType.bypass,
                        ins=[t2_in[0:AGH, :]],
                        outs=[t2_full[0:N_CORES * AGH, :]], replica_groups=RG)
            cc_t2b = nc.gpsimd.collective_compute(
                "AllGather", mybir.AluOpType.bypass,
                ins=[t2_in[AGH:SLICE, :]], outs=[t2_full[N_CORES * AGH:N, :]],
                replica_groups=RG)

            # ================= layer 2 (aggregate y2 @ width 256) =========
            # group-A gathers (sources in the first AG half) run LOOK windows
            # ahead so generation starts as soon as the first half lands.
            LOOK = 3
            t2_a = t2_full[0:N_CORES * AGH, :]
            l2_msgs = {}

            def l2_head(w_):
                msgs = msgsp.tile([128, CW, F2], BF16, tag="msgs")
                l2_msgs[w_] = msgs
                gi = gather(msgs, t2_a, gix2, w_, F2, 0, CWA)
                if w_ == 0:
                    tile_rust.add_dep_helper(
                        gi[0].ins, cc_t2b.ins, sync=False,
                        reason="order: trigger t2 half-2 AG before L2 A-heads")

            def l2_body(w_):
                r = rows_of(w_)
                msgs = l2_msgs.pop(w_)
                gather(msgs, t2_full[:], gix2, w_, F2, CWA, CW)
                y2self = selfp.tile([128, F2], BF16, tag="y2self")
                nc.sync.dma_start(y2self[:r, :],
                                  t2_in[w_ * W:w_ * W + r, :])
                hts = []
                for m in range(2):
                    pz = pzp.tile([128, W], F32, tag="pz")
                    for k in range(CW):
                        nc.tensor.matmul(pz[:], msgs[:, k, m * 128:(m + 1) * 128],
                                         s01[:, w_ * CW + k, :],
                                         start=(k == 0), stop=False)
                    nc.tensor.matmul(pz[:], y2self[:r, m * 128:(m + 1) * 128],
                                     diagcb[0:r, w_ * W:(w_ + 1) * W],
                                     start=False, stop=False)
                    nc.tensor.matmul(pz[:], b2[:, m * 128:(m + 1) * 128],
                                     win(recip, w_), start=False, stop=True)
                    ht = htp.tile([128, W], F32, tag="ht")
                    nc.scalar.activation(ht[:], pz[:],
                                         mybir.ActivationFunctionType.Relu)
                    hts.append(ht)
                py = pyp.tile([128, F3], F32, tag="py")
                for m in range(2):
                    nc.tensor.matmul(py[:], hts[m][:], w3[:, m, :],
                                     start=(m == 0), stop=(m == 1))
                y3 = evacp.tile([128, F3], BF16, tag="y3")
                nc.scalar.activation(y3[:], py[:],
                                     mybir.ActivationFunctionType.Copy,
                                     scale=dinv2T[:, w_:w_ + 1])
                nc.sync.dma_start(t3_in[w_ * W:w_ * W + r, 0:F3], y3[:r, :])
                if w_ == 4:
                    nc.gpsimd.collective_compute(
                        "AllGather", mybir.AluOpType.bypass,
                        ins=[t3_in[0:AGH, :]],
                        outs=[t3_full[0:N_CORES * AGH, :]], replica_groups=RG)

            for w_ in range(NW):
                l2_head(w_)
                if w_ >= LOOK:
                    l2_body(w_ - LOOK)
            for w_ in range(NW - LOOK, NW):
                l2_body(w_)
            cc_t3b = nc.gpsimd.collective_compute(
                "AllGather", mybir.AluOpType.bypass,
                ins=[t3_in[AGH:SLICE, :]], outs=[t3_full[N_CORES * AGH:N, :]],
                replica_groups=RG)

            # ================= layer 3 (aggregate y3 @ width 64) ==========
            t3ov_a = dataclasses.replace(
                t3_full[:], ap=[[2 * F3, N_CORES * AGH - 1], [1, 4 * F3]])
            t3ov = dataclasses.replace(t3_full[:], ap=[[2 * F3, N], [1, 4 * F3]])
            l3_msgs = {}

            def l3_head(w_):
                msgs = msgsp.tile([128, CW, 4 * F3], BF16, tag="msgs")
                l3_msgs[w_] = msgs
                gi = gather(msgs, t3ov_a, gix2, w_, 4 * F3, 0, CWA,
                            elem_step=2 * F3)
                if w_ == 0:
                    tile_rust.add_dep_helper(
                        gi[0].ins, cc_t3b.ins, sync=False,
                        reason="order: trigger t3 half-2 AG before L3 A-heads")

            def l3_body(w_):
                r = rows_of(w_)
                msgs = l3_msgs.pop(w_)
                gather(msgs, t3ov, gix2, w_, 4 * F3, CWA, CW,
                       elem_step=2 * F3)
                y3self = selfp.tile([128, F3], BF16, tag="y3self")
                nc.sync.dma_start(y3self[:r, :],
                                  t3_in[w_ * W:w_ * W + r, 0:F3])
                pz = pzp.tile([F3, W], F32, tag="pz")
                for k in range(CW):
                    nc.tensor.matmul(pz[:], msgs[:, k, 0:F3],
                                     s01[:, w_ * CW + k, :],
                                     start=(k == 0), stop=False)
                nc.tensor.matmul(pz[:], y3self[:r, :],
                                 diagcb[0:r, w_ * W:(w_ + 1) * W],
                                 start=False, stop=False)
                nc.tensor.matmul(pz[:], b3[:], win(recip, w_),
                                 start=False, stop=True)
                ht = htp.tile([F3, W], F32, tag="ht3")
                nc.scalar.activation(ht[:], pz[:],
                                     mybir.ActivationFunctionType.Relu)
                po = php.tile([128, FC], F32, tag="ph")
                nc.tensor.matmul(po[:], ht[:], wc[:], start=True, stop=False)
                nc.tensor.matmul(po[:], win(recip, w_), bc[:],
                                 start=False, stop=True)
                os_ = evacp.tile([128, FC], F32, tag="os")
                nc.scalar.activation(os_[:], po[:],
                                     mybir.ActivationFunctionType.Copy,
                                     scale=dinvT[:, w_:w_ + 1])
                nc.sync.dma_start(outd[w_ * W:w_ * W + r, :], os_[:r, :])

            for w_ in range(NW):
                l3_head(w_)
                if w_ >= LOOK:
                    l3_body(w_ - LOOK)
            for w_ in range(NW - LOOK, NW):
                l3_body(w_)

    nc.compile()
    return nc


# --------------------------------------------------------------------------
def kernel(x, W1, b1, W2, b2, W3, b3, Wc, bc, edge_index, _run_kwargs=None):
    x = np.asarray(x, dtype=np.float32)
    pre = _preprocess(np.asarray(edge_index))
    CW, NCH, EP = pre["CW"], pre["NCH"], pre["EP"]

    key = (CW, pre["CWA"])
    if key not in _cache:
        _cache[key] = _build(CW, pre["CWA"], NCH, EP)
    nc = _cache[key]

    common = {
        "x": x,
        "W1": np.asarray(W1, np.float32), "W2": np.asarray(W2, np.float32),
        "W3": np.asarray(W3, np.float32), "Wc": np.asarray(Wc, np.float32),
        "b1": np.asarray(b1, np.float32).reshape(1, F1),
        "b2": np.asarray(b2, np.float32).reshape(1, F2),
        "b3": np.asarray(b3, np.float32).reshape(1, F3),
        "bc": np.asarray(bc, np.float32).reshape(1, FC),
    }
    in_maps = []
    for c in range(N_CORES):
        m = dict(common)
        m["s01"] = pre["s01"][c].reshape(128, NCH * 128).astype(ml_dtypes.bfloat16)
        m["gidx"] = pre["gidx"][c]
        m["gidx2"] = pre["gidx2"][c]
        m["dsrc"] = pre["dsrc"][c]
        m["dinvT"] = pre["dinvT"][c]
        m["diag"] = pre["diag"][c]
        m["diagc"] = pre["diagc"][c]
        m["diagcb"] = pre["diagc"][c].astype(ml_dtypes.bfloat16)
        xs = np.zeros((NW * W, FEAT), np.float32)
        xs[:SLICE] = x[c * SLICE:(c + 1) * SLICE]
        m["xself"] = xs
        m["dinv2T"] = pre["dinv2T"][c]
        m["recip"] = pre["recip"][c]
        in_maps.append(m)

    kw = dict(_run_kwargs or {})
    res = run_bass_kernel_spmd(nc, in_maps, core_ids=list(range(N_CORES)), **kw)
    out = np.concatenate([res.results[c]["out"] for c in range(N_CORES)], axis=0)
    kernel._last_result = res
    return out


# revision 21
# speedup vs baseline: 1.0806x; 1.0261x over previous
"""3-layer GCN (ColorGNN) on 8 Trainium2 NeuronCores.

Strategy (sharding_hint: shard nodes + incident edges, replicate weights):
  - Each core owns a contiguous slice of 1250 dst nodes. Edges (incl.
    self-loops) are bucketed by dst into per-core windows of 128 dst nodes,
    padded on host to a uniform chunk grid so all 8 cores run one SPMD
    instruction stream.
  - GCN normalization dinv[s]*dinv[d] is folded into host-built one-hot
    scatter matrices (edge->dst-slot), dinv^2 scales at the table
    evacuations, and rank-1 bias matmuls, so the per-layer pipeline is:
        gather rows (dma_gather from a replicated DRAM table)
        -> PE scatter-matmul into PSUM (feature-major, no transposes)
        -> relu / weight matmuls -> write next table slice
        -> AllGather table across cores.
  - Aggregation happens at the cheap end of each layer: layer 1 aggregates
    x at width 128 before W1; layers 2/3 aggregate after W2/W3 at widths
    256/64 (linearity of the GCN propagation).
"""

import sys

if "/opt/trn_rl_repo" not in sys.path:
    sys.path.insert(0, "/opt/trn_rl_repo")

import dataclasses

import numpy as np
import ml_dtypes

import concourse.bacc as bacc
import concourse.mybir as mybir
import concourse.tile as tile
import concourse.tile_rust as tile_rust
from concourse.bass_utils import run_bass_kernel_spmd

# ---- problem constants (hardcoded per harness contract) ----
N = 10000
FEAT = 128
F1, F2, F3, FC = 512, 256, 64, 3
N_CORES = 8
SLICE = N // N_CORES          # 1250 dst nodes per core
W = 128                       # dst-window width (PSUM partition dim)
NW = (SLICE + W - 1) // W     # 10 windows; last is 98 wide
LAST_W = SLICE - (NW - 1) * W # 98

BF16 = mybir.dt.bfloat16
F32 = mybir.dt.float32
I16 = mybir.dt.int16

_cache = {}


# --------------------------------------------------------------------------
# host-side graph preprocessing (index/normalization structure only)
# --------------------------------------------------------------------------
def _preprocess(edge_index):
    src = np.asarray(edge_index[0], dtype=np.int64)
    dst = np.asarray(edge_index[1], dtype=np.int64)
    # self-loop edges are handled separately (contiguous local rows); drop
    # any explicit (i, i) duplicates from the edge list into the loop count.
    deg = np.bincount(dst, minlength=N).astype(np.float64) + 1.0
    dinv = 1.0 / np.sqrt(deg)

    keep = src != dst
    loop_extra = np.bincount(dst[~keep], minlength=N)  # explicit self-edges
    s, d = src[keep], dst[keep]

    core_of = d // SLICE
    win_of = (d % SLICE) // W
    order = np.lexsort((s, win_of, core_of))
    s, d = s[order], d[order]
    core_of, win_of = core_of[order], win_of[order]

    # per (core, window): dedup sources -> slots; S column = multi-hot counts
    run_key = (core_of * NW + win_of)
    run_starts = np.searchsorted(run_key, np.arange(N_CORES * NW))
    run_ends = np.append(run_starts[1:], len(s))

    # split each window's deduped sources into two groups by which half of
    # the split-layout tables (first 8*AGH rows vs rest) they live in, so
    # layer-2/3 gathers for group A can start after the first AllGather half.
    AGH = 640
    ACUT = N_CORES * AGH - 2   # last row is reserved for L3's 2-row reads

    def remap(g):
        gc, gi = g // SLICE, g % SLICE
        return np.where(gi < AGH, gc * AGH + gi,
                        N_CORES * AGH + gc * (SLICE - AGH) + (gi - AGH))

    slots_list = [[None] * NW for _ in range(N_CORES)]
    nslotA = np.zeros((N_CORES, NW), dtype=np.int64)
    nslotB = np.zeros((N_CORES, NW), dtype=np.int64)
    for c in range(N_CORES):
        for w_ in range(NW):
            k = c * NW + w_
            ss = s[run_starts[k]:run_ends[k]]
            dd = d[run_starts[k]:run_ends[k]]
            uniq, inv = np.unique(ss, return_inverse=True)
            is_a = remap(uniq) <= ACUT
            # stable reorder: A slots first, then B
            order_ = np.argsort(~is_a, kind="stable")
            rank = np.empty_like(order_)
            rank[order_] = np.arange(len(uniq))
            slots_list[c][w_] = (uniq[order_], rank[inv], dd, int(is_a.sum()))
            nslotA[c, w_] = is_a.sum()
            nslotB[c, w_] = len(uniq) - is_a.sum()
    CWA = int(np.max((nslotA + 127) // 128))
    CWB = int(np.max((nslotB + 127) // 128))
    CW = CWA + CWB
    EPW = CW * 128
    NCH = NW * CW
    EP = NCH * 128

    g_src = np.zeros((N_CORES, EP), dtype=np.int64)
    s01 = np.zeros((N_CORES, 128, NCH, 128), dtype=np.float32)
    dsrc = np.zeros((N_CORES, 128, NCH), dtype=np.float32)
    for c in range(N_CORES):
        for w_ in range(NW):
            uniq, inv, dd, na = slots_list[c][w_]
            base = w_ * EPW
            # slot position: A slots at [0, na), B slots at [CWA*128, ...)
            pos = np.arange(len(uniq))
            pos = np.where(pos < na, pos, CWA * 128 + (pos - na))
            np.put(g_src[c], base + pos, uniq)
            slot = base + pos[inv]
            part = slot % 128
            chunk = slot // 128
            dstl = (dd % SLICE) - w_ * W
            np.add.at(s01[c], (part, chunk, dstl), 1.0)
            sl = base + pos
            dsrc[c, sl % 128, sl // 128] = dinv[uniq]

    # gather idx layouts: wrapped-16 int16, one raw (L1/x) one remapped
    g2 = remap(g_src)
    gidx = np.zeros((N_CORES, 128, EP // 16), dtype=np.int16)
    gidx2 = np.zeros((N_CORES, 128, EP // 16), dtype=np.int16)
    for c in range(N_CORES):
        gidx[c] = np.tile(g_src[c].astype(np.int16).reshape(-1, 16).T, (8, 1))
        gidx2[c] = np.tile(g2[c].astype(np.int16).reshape(-1, 16).T, (8, 1))

    # per-window per-dst-node vectors
    dinv_pad = np.zeros((N_CORES, NW * W), dtype=np.float64)
    recip_pad = np.zeros((N_CORES, NW * W), dtype=np.float64)
    for c in range(N_CORES):
        sl = dinv[c * SLICE:(c + 1) * SLICE]
        dinv_pad[c, :SLICE] = sl
        recip_pad[c, :SLICE] = 1.0 / sl
    dinvT = np.ascontiguousarray(
        dinv_pad.reshape(N_CORES, NW, W).transpose(0, 2, 1)).astype(np.float32)
    dinv2T = np.ascontiguousarray(
        (dinv_pad ** 2).reshape(N_CORES, NW, W).transpose(0, 2, 1)).astype(np.float32)
    recip_row = recip_pad.astype(np.float32).reshape(N_CORES, 1, NW * W)

    # self-loop diagonal (value = dinv[n] * loop multiplicity incl implicit)
    diag = np.zeros((N_CORES, 128, NW * W), dtype=np.float32)
    loopv = dinv * (1.0 + loop_extra)
    for c in range(N_CORES):
        for w_ in range(NW):
            r = W if w_ < NW - 1 else LAST_W
            rows = np.arange(r)
            diag[c, rows, w_ * W + rows] = loopv[c * SLICE + w_ * W:
                                                 c * SLICE + w_ * W + r]
    # loop multiplicity for L2/L3 identity paths (value = multiplicity)
    diagc = np.zeros((N_CORES, 128, NW * W), dtype=np.float32)
    for c in range(N_CORES):
        for w_ in range(NW):
            r = W if w_ < NW - 1 else LAST_W
            rows = np.arange(r)
            diagc[c, rows, w_ * W + rows] = (
                1.0 + loop_extra[c * SLICE + w_ * W:c * SLICE + w_ * W + r])

    return dict(CW=CW, CWA=CWA, NCH=NCH, EP=EP, s01=s01, gidx=gidx,
                gidx2=gidx2, dsrc=dsrc, dinvT=dinvT, dinv2T=dinv2T,
                recip=recip_row, diag=diag, diagc=diagc)


# --------------------------------------------------------------------------
# device graph (one SPMD program for all 8 cores)
# --------------------------------------------------------------------------
def _build(CW, CWA, NCH, EP):
    # default 16KB SWDGE descriptor carveout -> 1024-descriptor ring per
    # queue; gather calls are split into <=GS-chunk pieces (GS*128
    # descriptors) and alternate between 2 queues so two stay in flight.
    nc = bacc.Bacc("TRN2", target_bir_lowering=False, debug=False,
                   num_swdge_queues=2)
    GS = 8
    AGH = 5 * W   # all-gather first-half rows (windows 0-4)

    xin = nc.dram_tensor("x", [N, FEAT], F32, kind="ExternalInput")
    w1d = nc.dram_tensor("W1", [FEAT, F1], F32, kind="ExternalInput")
    w2d = nc.dram_tensor("W2", [F1, F2], F32, kind="ExternalInput")
    w3d = nc.dram_tensor("W3", [F2, F3], F32, kind="ExternalInput")
    wcd = nc.dram_tensor("Wc", [F3, FC], F32, kind="ExternalInput")
    b1d = nc.dram_tensor("b1", [1, F1], F32, kind="ExternalInput")
    b2d = nc.dram_tensor("b2", [1, F2], F32, kind="ExternalInput")
    b3d = nc.dram_tensor("b3", [1, F3], F32, kind="ExternalInput")
    bcd = nc.dram_tensor("bc", [1, FC], F32, kind="ExternalInput")
    s01d = nc.dram_tensor("s01", [128, NCH * 128], BF16, kind="ExternalInput")
    gixd = nc.dram_tensor("gidx", [128, EP // 16], I16, kind="ExternalInput")
    gixd2 = nc.dram_tensor("gidx2", [128, EP // 16], I16, kind="ExternalInput")
    dsrcd = nc.dram_tensor("dsrc", [128, NCH], F32, kind="ExternalInput")
    dinvTd = nc.dram_tensor("dinvT", [128, NW], F32, kind="ExternalInput")
    dinv2Td = nc.dram_tensor("dinv2T", [128, NW], F32, kind="ExternalInput")
    recipd = nc.dram_tensor("recip", [1, NW * W], F32, kind="ExternalInput")
    diagd = nc.dram_tensor("diag", [128, NW * W], F32, kind="ExternalInput")
    diagcd = nc.dram_tensor("diagc", [128, NW * W], F32, kind="ExternalInput")
    diagcbd = nc.dram_tensor("diagcb", [128, NW * W], BF16, kind="ExternalInput")
    xselfd = nc.dram_tensor("xself", [NW * W, FEAT], F32, kind="ExternalInput")
    outd = nc.dram_tensor("out", [SLICE, FC], F32, kind="ExternalOutput")
    # +16 pad rows: L3 gathers read two consecutive 128-elem bf16 rows
    # (512B descriptors run ~2.7x faster than 256B), discarding the pad
    # halves; bf16 rows let the scatter use s01 directly (no DVE cast,
    # which locks GpSimd out of the SWDGE descriptor rings).
    t2_full = nc.dram_tensor("t2_full", [N, F2], BF16, kind="Internal",
                             addr_space="Shared")
    t3_full = nc.dram_tensor("t3_full", [N + 16, 2 * F3], BF16,
                             kind="Internal", addr_space="Shared")

    RG = [list(range(N_CORES))]

    with tile.TileContext(nc) as tc:
        with (
            tc.tile_pool(name="res", bufs=1) as res,
            tc.tile_pool(name="msgs", bufs=8) as msgsp,
            tc.tile_pool(name="smat", bufs=2) as smatp,
            tc.tile_pool(name="ht", bufs=6) as htp,
            tc.tile_pool(name="evac", bufs=3) as evacp,
            tc.tile_pool(name="selfp", bufs=3) as selfp,
            tc.tile_pool(name="pz", bufs=2, space="PSUM") as pzp,
            tc.tile_pool(name="ph", bufs=3, space="PSUM") as php,
            tc.tile_pool(name="py", bufs=2, space="PSUM") as pyp,
            tc.tile_pool(name="dram", bufs=1, space="DRAM") as dram,
        ):
            # ---- PE warm-up: dense dummy matmuls so HAM unthrottles before
            # layer 1 (overlaps the input loads / entry barrier) ----
            wu = res.tile([128, 512], F32)
            nc.gpsimd.memset(wu[:], 1.0)
            wups = pyp.tile([128, 512], F32, tag="py")
            for i in range(14):
                nc.tensor.matmul(wups[:], wu[:, 0:128], wu[:],
                                 start=(i == 0), stop=(i == 13))
            wuev = res.tile([128, 4], F32)
            nc.vector.tensor_copy(wuev[:], wups[:, 0:4])
            nc.sync.dma_start(outd[0:128, 0:3], wuev[:, 0:3])

            # ---- resident loads (gather idxs first: L1 gathers gate on them)
            gix = res.tile([128, EP // 16], I16)
            nc.sync.dma_start(gix[:], gixd[:])
            gix2 = res.tile([128, EP // 16], I16)
            nc.sync.dma_start(gix2[:], gixd2[:])
            s01 = res.tile([128, NCH, 128], BF16)
            nc.sync.dma_start(s01[:], s01d[:].rearrange("p (c j) -> p c j", j=128))
            dsrc = res.tile([128, NCH], F32)
            nc.sync.dma_start(dsrc[:], dsrcd[:])
            dinvT = res.tile([128, NW], F32)
            nc.sync.dma_start(dinvT[:], dinvTd[:])
            dinv2T = res.tile([128, NW], F32)
            nc.sync.dma_start(dinv2T[:], dinv2Td[:])
            recip = res.tile([1, NW * W], F32)
            nc.sync.dma_start(recip[:], recipd[:])
            diag = res.tile([128, NW * W], F32)
            nc.sync.dma_start(diag[:], diagd[:])
            diagc = res.tile([128, NW * W], F32)
            nc.sync.dma_start(diagc[:], diagcd[:])
            diagcb = res.tile([128, NW * W], BF16)
            nc.sync.dma_start(diagcb[:], diagcbd[:])
            w1 = res.tile([128, F1], F32)
            nc.sync.dma_start(w1[:], w1d[:])
            w2 = res.tile([128, 4, F2], F32)
            nc.sync.dma_start(w2[:], w2d[:].rearrange("(c p) f -> p c f", p=128))
            w3 = res.tile([128, 2, F3], F32)
            nc.sync.dma_start(w3[:], w3d[:].rearrange("(c p) f -> p c f", p=128))
            wc = res.tile([F3, FC], F32)
            nc.sync.dma_start(wc[:], wcd[:])
            b1 = res.tile([1, F1], F32)
            nc.sync.dma_start(b1[:], b1d[:])
            b2 = res.tile([1, F2], F32)
            nc.sync.dma_start(b2[:], b2d[:])
            b3 = res.tile([1, F3], F32)
            nc.sync.dma_start(b3[:], b3d[:])
            bc = res.tile([1, FC], F32)
            nc.sync.dma_start(bc[:], bcd[:])

            # ---- internal DRAM tables ----
            t2_in = dram.tile([NW * W, F2], BF16)
            t3_in = dram.tile([NW * W, 2 * F3], BF16)

            qctr = [0]

            def gather(dst_tile, table_ap, idx_tile, w_, elem, c0=0, c1=None,
                       elem_step=None):
                insts = []
                if c1 is None:
                    c1 = CW
                for a in range(c0, c1, GS):
                    b = min(a + GS, c1)
                    n_ = (b - a) * 128
                    insts.append(nc.gpsimd.dma_gather(
                        dst_tile[:, a:b, :], table_ap,
                        idx_tile[:, (w_ * CW + a) * 8:(w_ * CW + b) * 8],
                        n_, n_, elem, elem_step=elem_step,
                        queue_num=qctr[0] % 2))
                    qctr[0] += 1
                return insts

            def rows_of(w_):
                return W if w_ < NW - 1 else LAST_W

            def win(t, w_):  # [1, W] slice of a [1, NW*W] row vector
                return t[:, w_ * W:(w_ + 1) * W]

            # ================= layer 1 (aggregate x @ width 128) ==========
            for w_ in range(NW):
                r = rows_of(w_)
                msgs = msgsp.tile([128, CW, FEAT], F32, tag="msgs")
                gather(msgs, xin[:], gix, w_, FEAT)
                s1 = smatp.tile([128, CW, 128], F32, tag="smat")
                nc.vector.tensor_tensor(
                    s1[:], s01[:, w_ * CW:(w_ + 1) * CW, :],
                    dsrc[:, w_ * CW:(w_ + 1) * CW].to_broadcast((128, CW, 128)),
                    mybir.AluOpType.mult)
                xself = selfp.tile([128, FEAT], F32, tag="xself")
                nc.sync.dma_start(
                    xself[:r, :], xselfd[w_ * W:w_ * W + r, :])
                pz = pzp.tile([128, W], F32, tag="pz")
                for k in range(CW):
                    nc.tensor.matmul(pz[:], msgs[:, k, :], s1[:, k, :],
                                     start=(k == 0), stop=False)
                nc.tensor.matmul(pz[:], xself[:r, :],
                                 diag[0:r, w_ * W:(w_ + 1) * W],
                                 start=False, stop=True)
                z1 = evacp.tile([128, W], F32, tag="z1")
                nc.vector.tensor_copy(z1[:], pz[:])

                hts = []
                for c4 in range(4):
                    ph = php.tile([128, W], F32, tag="ph")
                    nc.tensor.matmul(ph[:], w1[:, c4 * 128:(c4 + 1) * 128], z1[:],
                                     start=True, stop=False)
                    nc.tensor.matmul(ph[:], b1[:, c4 * 128:(c4 + 1) * 128],
                                     win(recip, w_), start=False, stop=True)
                    ht = htp.tile([128, W], F32, tag="ht")
                    nc.scalar.activation(ht[:], ph[:],
                                         mybir.ActivationFunctionType.Relu)
                    hts.append(ht)
                py = pyp.tile([128, F2], F32, tag="py")
                for c4 in range(4):
                    nc.tensor.matmul(py[:], hts[c4][:], w2[:, c4, :],
                                     start=(c4 == 0), stop=(c4 == 3))
                y2 = evacp.tile([128, F2], BF16, tag="y2")
                nc.scalar.activation(y2[:], py[:],
                                     mybir.ActivationFunctionType.Copy,
                                     scale=dinv2T[:, w_:w_ + 1])
                nc.sync.dma_start(t2_in[w_ * W:w_ * W + r, :], y2[:r, :])
                if w_ == 7:
                    nc.gpsimd.collective_compute(
                        "AllGather", mybir.AluOpType.bypass,
                        ins=[t2_in[0:AGH, :]],
                        outs=[t2_full[0:N_CORES * AGH, :]], replica_groups=RG)
            cc_t2b = nc.gpsimd.collective_compute(
                "AllGather", mybir.AluOpType.bypass,
                ins=[t2_in[AGH:SLICE, :]], outs=[t2_full[N_CORES * AGH:N, :]],
                replica_groups=RG)

            # ================= layer 2 (aggregate y2 @ width 256) =========
            # group-A gathers (sources in the first AG half) run LOOK windows
            # ahead so generation starts as soon as the first half lands.
            LOOK = 3
            t2_a = t2_full[0:N_CORES * AGH, :]
            l2_msgs = {}

            def l2_head(w_):
                msgs = msgsp.tile([128, CW, F2], BF16, tag="msgs")
                l2_msgs[w_] = msgs
                gi = gather(msgs, t2_a, gix2, w_, F2, 0, CWA)
                if w_ == 0:
                    tile_rust.add_dep_helper(
                        gi[0].ins, cc_t2b.ins, sync=False,
                        reason="order: trigger t2 half-2 AG before L2 A-heads")

            def l2_body(w_):
                r = rows_of(w_)
                msgs = l2_msgs.pop(w_)
                gather(msgs, t2_full[:], gix2, w_, F2, CWA, CW)
                y2self = selfp.tile([128, F2], BF16, tag="y2self")
                nc.sync.dma_start(y2self[:r, :],
                                  t2_in[w_ * W:w_ * W + r, :])
                hts = []
                for m in range(2):
                    pz = pzp.tile([128, W], F32, tag="pz")
                    for k in range(CW):
                        nc.tensor.matmul(pz[:], msgs[:, k, m * 128:(m + 1) * 128],
                                         s01[:, w_ * CW + k, :],
                                         start=(k == 0), stop=False)
                    nc.tensor.matmul(pz[:], y2self[:r, m * 128:(m + 1) * 128],
                                     diagcb[0:r, w_ * W:(w_ + 1) * W],
                                     start=False, stop=False)
                    nc.tensor.matmul(pz[:], b2[:, m * 128:(m + 1) * 128],
                                     win(recip, w_), start=False, stop=True)
                    ht = htp.tile([128, W], F32, tag="ht")
                    nc.scalar.activation(ht[:], pz[:],
                                         mybir.ActivationFunctionType.Relu)
                    hts.append(ht)
                py = pyp.tile([128, F3], F32, tag="py")
                for m in range(2):
                    nc.tensor.matmul(py[:], hts[m][:], w3[:, m, :],
                                     start=(m == 0), stop=(m == 1))
                y3 = evacp.tile([128, F3], BF16, tag="y3")
                nc.scalar.activation(y3[:], py[:],
                                     mybir.ActivationFunctionType.Copy,
                                     scale=dinv2T[:, w_:w_ + 1])
                nc.sync.dma_start(t3_in[w_ * W:w_ * W + r, 0:F3], y3[:r, :])
                if w_ == 4:
                    nc.gpsimd.collective_compute(
                        "AllGather", mybir.AluOpType.bypass,
                        ins=[t3_in[0:AGH, :]],
                        outs=[t3_full[0:N_CORES * AGH, :]], replica_groups=RG)

            for w_ in range(NW):
                l2_head(w_)
                if w_ >= LOOK:
                    l2_body(w_ - LOOK)
            for w_ in range(NW - LOOK, NW):
                l2_body(w_)
            cc_t3b = nc.gpsimd.collective_compute(
                "AllGather", mybir.AluOpType.bypass,
                ins=[t3_in[AGH:SLICE, :]], outs=[t3_full[N_CORES * AGH:N, :]],
                replica_groups=RG)

            # ================= layer 3 (aggregate y3 @ width 64) ==========
            t3ov_a = dataclasses.replace(
                t3_full[:], ap=[[2 * F3, N_CORES * AGH - 1], [1, 4 * F3]])
            t3ov = dataclasses.replace(t3_full[:], ap=[[2 * F3, N], [1, 4 * F3]])
            l3_msgs = {}

            def l3_head(w_):
                msgs = msgsp.tile([128, CW, 4 * F3], BF16, tag="msgs")
                l3_msgs[w_] = msgs
                gi = gather(msgs, t3ov_a, gix2, w_, 4 * F3, 0, CWA,
                            elem_step=2 * F3)
                if w_ == 0:
                    tile_rust.add_dep_helper(
                        gi[0].ins, cc_t3b.ins, sync=False,
                        reason="order: trigger t3 half-2 AG before L3 A-heads")

            def l3_body(w_):
                r = rows_of(w_)
                msgs = l3_msgs.pop(w_)
                gather(msgs, t3ov, gix2, w_, 4 * F3, CWA, CW,
                       elem_step=2 * F3)
                y3self = selfp.tile([128, F3], BF16, tag="y3self")
                nc.sync.dma_start(y3self[:r, :],
                                  t3_in[w_ * W:w_ * W + r, 0:F3])
                pz = pzp.tile([F3, W], F32, tag="pz")
                for k in range(CW):
                    nc.tensor.matmul(pz[:], msgs[:, k, 0:F3],
                                     s01[:, w_ * CW + k, :],
                                     start=(k == 0), stop=False)
                nc.tensor.matmul(pz[:], y3self[:r, :],
                                 diagcb[0:r, w_ * W:(w_ + 1) * W],
                                 start=False, stop=False)
                nc.tensor.matmul(pz[:], b3[:], win(recip, w_),
                                 start=False, stop=True)
                ht = htp.tile([F3, W], F32, tag="ht3")
                nc.scalar.activation(ht[:], pz[:],
                                     mybir.ActivationFunctionType.Relu)
                po = php.tile([128, FC], F32, tag="ph")
                nc.tensor.matmul(po[:], ht[:], wc[:], start=True, stop=False)
                nc.tensor.matmul(po[:], win(recip, w_), bc[:],
                                 start=False, stop=True)
                os_ = evacp.tile([128, FC], F32, tag="os")
                nc.scalar.activation(os_[:], po[:],
                                     mybir.ActivationFunctionType.Copy,
                                     scale=dinvT[:, w_:w_ + 1])
                nc.sync.dma_start(outd[w_ * W:w_ * W + r, :], os_[:r, :])

            for w_ in range(NW):
                l3_head(w_)
                if w_ >= LOOK:
                    l3_body(w_ - LOOK)
            for w_ in range(NW - LOOK, NW):
                l3_body(w_)

    nc.compile()
    return nc


# --------------------------------------------------------------------------
def kernel(x, W1, b1, W2, b2, W3, b3, Wc, bc, edge_index, _run_kwargs=None):
    x = np.asarray(x, dtype=np.float32)
    pre = _preprocess(np.asarray(edge_index))
    CW, NCH, EP = pre["CW"], pre["NCH"], pre["EP"]

    key = (CW, pre["CWA"])
    if key not in _cache:
        _cache[key] = _build(CW, pre["CWA"], NCH, EP)
    nc = _cache[key]

    common = {
        "x": x,
        "W1": np.asarray(W1, np.float32), "W2": np.asarray(W2, np.float32),
        "W3": np.asarray(W3, np.float32), "Wc": np.asarray(Wc, np.float32),
        "b1": np.asarray(b1, np.float32).reshape(1, F1),
        "b2": np.asarray(b2, np.float32).reshape(1, F2),
        "b3": np.asarray(b3, np.float32).reshape(1, F3),
        "bc": np.asarray(bc, np.float32).reshape(1, FC),
    }
    in_maps = []
    for c in range(N_CORES):
        m = dict(common)
        m["s01"] = pre["s01"][c].reshape(128, NCH * 128).astype(ml_dtypes.bfloat16)
        m["gidx"] = pre["gidx"][c]
        m["gidx2"] = pre["gidx2"][c]
        m["dsrc"] = pre["dsrc"][c]
        m["dinvT"] = pre["dinvT"][c]
        m["diag"] = pre["diag"][c]
        m["diagc"] = pre["diagc"][c]
        m["diagcb"] = pre["diagc"][c].astype(ml_dtypes.bfloat16)
        xs = np.zeros((NW * W, FEAT), np.float32)
        xs[:SLICE] = x[c * SLICE:(c + 1) * SLICE]
        m["xself"] = xs
        m["dinv2T"] = pre["dinv2T"][c]
        m["recip"] = pre["recip"][c]
        in_maps.append(m)

    kw = dict(_run_kwargs or {})
    res = run_bass_kernel_spmd(nc, in_maps, core_ids=list(range(N_CORES)), **kw)
    out = np.concatenate([res.results[c]["out"] for c in range(N_CORES)], axis=0)
    kernel._last_result = res
    return out
